# revision 78
# baseline (speedup 1.0000x reference)
"""Trainium2 Bass kernel for per-sample dynamic conv (SE-attention mixed 3x3 kernels).

Computation (per sample b):
    att[b, :]  = sigmoid(gn(mish(gn(mean_hw(x[b]) @ w1.T)) @ w2.T))   # [K]
    agg_w[b]   = sum_k att[b,k] * weight[k]                           # [C,C,3,3]
    agg_b[b]   = att[b, :] @ bias                                     # [C]
    out[b]     = conv2d(x[b], agg_w[b], padding=1) + agg_b[b]

Strategy: pure data parallel over batch on 8 NeuronCores (4 samples each).
Conv is done as 9 shifted bf16 matmuls per 8-row output block accumulating in
PSUM; x is staged host-side zero-padded to 66-wide rows so the shifts never
need edge fixups. Weights are staged host-side pre-transposed to [ci, tap, co]
(matmul lhsT layout). SE path runs in f32 on-chip.
"""

import os
import numpy as np
import ml_dtypes

BS, C, HH, WW = 32, 128, 64, 64
K, HID = 4, 8
N_CORES = 8
BSL = BS // N_CORES          # samples per core
LROW = WW + 2                # 66: row pitch with left/right zero pad
LPAD = HH * LROW + 2         # 4226: padded per-channel length (leading+trailing zero)
NTAP = 9
RB = 8                       # output rows per PSUM block
NBLK = HH // RB              # 8 blocks
EPS = 1e-5
BLOB_W = 256                 # packed small-param blob width (f32 columns)

_cache = {}

# exec time (ns) of the last hardware run, when tracing was enabled
LAST_EXEC_NS = None


def _install_trace_hook():
    """Make run_bass_kernel_spmd(trace=True) work under axon by supplying the
    missing antenv.axon_hooks module (NTFF profiling via libaxon ctypes)."""
    import sys, types
    if "antenv.axon_hooks" in sys.modules:
        return True
    try:
        from trn_agent_boot.trn_boot import _ntff_profile_via_ctypes
        hook = _ntff_profile_via_ctypes("/opt/axon/libaxon_pjrt.so")
    except Exception:
        return False
    m = types.ModuleType("antenv.axon_hooks")
    m.get_axon_ntff_profile_hook = lambda: hook
    m.set_axon_ntff_profile_hook = lambda h: None
    sys.modules["antenv.axon_hooks"] = m
    return True


def _build_nc():
    import concourse.bass as bass
    import concourse.tile as tile
    from concourse import bacc, mybir

    f32 = mybir.dt.float32
    bf16 = mybir.dt.bfloat16
    Alu = mybir.AluOpType
    Act = mybir.ActivationFunctionType

    nc = bacc.Bacc("TRN2", target_bir_lowering=False, debug=False,
                   enable_asserts=True, num_devices=N_CORES)

    xp_d = nc.dram_tensor("xp", [BSL, C, LPAD], bf16, kind="ExternalInput").ap()
    wt_d = nc.dram_tensor("wt", [K, C, NTAP * C], bf16, kind="ExternalInput").ap()
    blob_d = nc.dram_tensor("blob", [C, BLOB_W], f32, kind="ExternalInput").ap()
    out_d = nc.dram_tensor("out", [BSL, C, HH * WW], bf16, kind="ExternalOutput").ap()

    HB = BSL // 2   # samples per SE half-batch
    W1 = NTAP * C   # 1152 free elems per base kernel

    with tile.TileContext(nc) as tc:
        with (
            tc.tile_pool(name="xin", bufs=BSL) as xpool,
            tc.tile_pool(name="wts", bufs=1) as wpool,
            tc.tile_pool(name="small", bufs=1) as spool,
            tc.tile_pool(name="agg", bufs=1) as apool,
            tc.tile_pool(name="ostage", bufs=4) as opool,
            tc.tile_pool(name="psmall", bufs=2, space="PSUM") as pspool,
            tc.tile_pool(name="pconv", bufs=6, space="PSUM") as cpool,
        ):
            # ---- input DMA ----
            # Every dma_start costs ~1.25us of issue time on its sequencer and
            # one queue moves only ~90GB/s, so: few, fat, well-placed issues.
            # Samples 0/1 split 4 ways across sync+scalar (SE head latency);
            # samples 2/3 one issue each; weights and the packed param blob
            # one issue each. gpsimd must not issue DMA (its SWDGE descriptor
            # work locks the shared GpSimd/DVE SBUF port and stalls DVE).
            x_tiles = [xpool.tile([C, LPAD], bf16, tag=f"x{b}", name=f"xt{b}")
                       for b in range(BSL)]
            for b in (0, 1):
                for i, c in enumerate(range(0, C, 32)):
                    eng = nc.sync if i < 2 else nc.scalar
                    eng.dma_start(x_tiles[b][c:c + 32, :], xp_d[b][c:c + 32, :])
            for b in (2, 3):
                nc.sync.dma_start(x_tiles[b][:], xp_d[b])

            blob = spool.tile([C, BLOB_W], f32, tag="blob", name="blob")
            nc.scalar.dma_start(blob[:], blob_d[:])
            se1_sb = blob[:, 0:8]
            bias_sb = blob[0:K, 8:136]
            se2_sb = blob[0:HID, 136:140]
            eye_sb = blob[0:K, 140:144]
            gn1s_sb = blob[0:1, 144:152]
            gn1b_sb = blob[0:1, 152:160]
            gn2s_sb = blob[0:1, 160:164]
            gn2b_sb = blob[0:1, 164:168]

            # all 4 base kernels in one strided DMA: [k][ci][1152] -> [ci][k*1152]
            wt_all = wpool.tile([C, K * W1], bf16)
            wt_src = wt_d.transpose([1, 0, 2])
            nc.sync.dma_start(wt_all[:].rearrange("p (k w) -> p k w", k=K),
                              wt_src)

            # ---- pooling: pooled[ci, b] = sum_hw x[b, ci] ----
            # (zero padding doesn't affect the sum; the 1/4096 mean factor is
            # folded into se1 host-side). Samples 0/1: DVE+ACT halves (lowest
            # latency); samples 2/3: ACT-only (keeps DVE free for agg work).
            HALF = LPAD // 2
            # one tile per partial sum: Tile tracks deps per-TILE, so sharing
            # a [C,2] tile between samples makes sample 0's SE falsely wait
            # on sample 1's reduce
            p_dve0 = spool.tile([C, 1], f32, tag="p_dve0", name="p_dve0")
            p_act0 = spool.tile([C, 1], f32, tag="p_act0", name="p_act0")
            p_dve1 = spool.tile([C, 1], f32, tag="p_dve1", name="p_dve1")
            p_act1 = spool.tile([C, 1], f32, tag="p_act1", name="p_act1")
            p123 = spool.tile([C, 3], f32, tag="p123", name="p123")

            def pool_sample_split(b):
                pd = p_dve0 if b == 0 else p_dve1
                pa = p_act0 if b == 0 else p_act1
                r1 = nc.vector.tensor_reduce(
                    out=pd[:], in_=x_tiles[b][:, 0:HALF],
                    axis=mybir.AxisListType.X, op=Alu.add)
                r2 = nc.scalar.activation(x_tiles[b][:, HALF:LPAD],
                                          x_tiles[b][:, HALF:LPAD],
                                          Act.Identity, accum_out=pa[:])
                return r1, r2

            # sample 1's DVE half in four ~0.55us pieces: the scheduler will
            # slot them into SE0's ACT-hop gaps on DVE instead of blocking
            # the chain once for 2.35us
            p1v = spool.tile([C, 4], f32, tag="p1v", name="p1v")

            def pool_sample1():
                bounds = [0, 528, 1056, 1584, HALF]
                for i in range(4):
                    nc.vector.tensor_reduce(
                        out=p1v[:, i:i + 1],
                        in_=x_tiles[1][:, bounds[i]:bounds[i + 1]],
                        axis=mybir.AxisListType.X, op=Alu.add)
                nc.scalar.activation(x_tiles[1][:, HALF:LPAD],
                                     x_tiles[1][:, HALF:LPAD],
                                     Act.Identity, accum_out=p_act1[:])
                pm = spool.tile([C, 1], f32, tag="p1m", name="p1m")
                nc.vector.tensor_reduce(out=pm[:], in_=p1v[:],
                                        axis=mybir.AxisListType.X, op=Alu.add)
                nc.vector.tensor_add(p123[:, 0:1], pm[:], p_act1[:])

            def pool_sample_act(b):
                return nc.scalar.activation(x_tiles[b][:], x_tiles[b][:],
                                            Act.Identity,
                                            accum_out=p123[:, b - 1:b])

            def groupnorm_rows(h_ps, rows, n, scale_sb, bias_sb_, tag):
                """GroupNorm(1) over the free dim of a [rows, n] PSUM tile ->
                normalized+affine [rows, n] f32 SBUF tile. rstd = sqrt(1/s)
                via DVE reciprocal + globally-convergent Newton (no sqrt ACT
                table -> the whole SE stays on the one exp/tanh table)."""
                msum = spool.tile([rows, 1], f32, tag=f"{tag}_msum",
                                  name=f"{tag}_msum")
                nc.vector.tensor_reduce(out=msum[:], in_=h_ps[:],
                                        axis=mybir.AxisListType.X, op=Alu.add)
                mean = spool.tile([rows, 1], f32, tag=f"{tag}_mean",
                                  name=f"{tag}_mean")
                nc.vector.tensor_scalar_mul(mean[:], msum[:], 1.0 / n)
                cent = spool.tile([rows, n], f32, tag=f"{tag}_cent",
                                  name=f"{tag}_cent")
                nc.vector.tensor_scalar_sub(cent[:], h_ps[:], mean[:, 0:1])
                sq = spool.tile([rows, n], f32, tag=f"{tag}_sq",
                                name=f"{tag}_sq")
                vsum = spool.tile([rows, 1], f32, tag=f"{tag}_vsum",
                                  name=f"{tag}_vsum")
                nc.scalar.activation(sq[:], cent[:], Act.Square,
                                     accum_out=vsum[:])
                s = spool.tile([rows, 1], f32, tag=f"{tag}_s", name=f"{tag}_s")
                nc.vector.tensor_scalar(s[:], vsum[:], 1.0 / n, EPS,
                                        op0=Alu.mult, op1=Alu.add)
                r = spool.tile([rows, 1], f32, tag=f"{tag}_r", name=f"{tag}_r")
                nc.vector.reciprocal(r[:], s[:])
                rh = spool.tile([rows, 1], f32, tag=f"{tag}_rh",
                                name=f"{tag}_rh")
                nc.vector.tensor_scalar_mul(rh[:], r[:], 0.5)
                z = spool.tile([rows, 1], f32, tag=f"{tag}_z", name=f"{tag}_z")
                nc.vector.tensor_scalar(z[:], r[:], 1.0 / 16, 1.0,
                                        op0=Alu.mult, op1=Alu.max)
                for it in range(4):
                    u = spool.tile([rows, 1], f32, tag=f"{tag}_u{it}",
                                   name=f"{tag}_u{it}")
                    nc.vector.reciprocal(u[:], z[:])
                    tm = spool.tile([rows, 1], f32, tag=f"{tag}_tm{it}",
                                    name=f"{tag}_tm{it}")
                    nc.vector.tensor_mul(tm[:], rh[:], u[:])
                    zn = spool.tile([rows, 1], f32, tag=f"{tag}_zn{it}",
                                    name=f"{tag}_zn{it}")
                    nc.vector.scalar_tensor_tensor(
                        out=zn[:], in0=z[:], scalar=0.5, in1=tm[:],
                        op0=Alu.mult, op1=Alu.add)
                    z = zn
                # fused normalize+affine: (cent * rstd) * scale_bc + bias_bc
                s_bc = spool.tile([rows, n], f32, tag=f"{tag}_sbc",
                                  name=f"{tag}_sbc")
                nc.gpsimd.partition_broadcast(s_bc[:], scale_sb)
                b_bc = spool.tile([rows, n], f32, tag=f"{tag}_bbc",
                                  name=f"{tag}_bbc")
                nc.gpsimd.partition_broadcast(b_bc[:], bias_sb_)
                na = spool.tile([rows, n], f32, tag=f"{tag}_na",
                                name=f"{tag}_na")
                nc.vector.scalar_tensor_tensor(
                    out=na[:], in0=cent[:], scalar=z[:, 0:1], in1=s_bc[:],
                    op0=Alu.mult, op1=Alu.mult)
                out = spool.tile([rows, n], f32, tag=f"{tag}_out",
                                 name=f"{tag}_out")
                nc.vector.tensor_add(out[:], na[:], b_bc[:])
                return out

            # se_w2 rows broadcast once to 4 partitions (for the DVE dot
            # products that replace the mid-chain PE transpose + matmul)
            se2_bc = spool.tile([BSL, K * HID], f32, tag="se2bc",
                                name="se2bc")
            nc.gpsimd.partition_broadcast(se2_bc[:], blob[0:1, 200:232])
            biasT_sb = blob[:, 176:180]   # bias.T [C, K]

            def se_chain(rows, pooled_pieces, tag):
                """SE attention for `rows` samples batched on partitions.
                pooled_pieces: list of (row, [lhsT col APs]) — each row's
                pooled vector is the SUM of its pieces, accumulated directly
                in the h1 matmul (PSUM) so no DVE merge is ever needed.
                After the leading h1 matmul the chain never touches PE, so
                conv matmul streams issued later are never blocked by it.
                Returns att [rows, K] f32 SBUF."""
                h1_ps = pspool.tile([rows, HID], f32, tag="seps",
                                    name=f"{tag}h1")
                for row, pieces in pooled_pieces:
                    nrow = pieces[0].free_size()
                    for pi, ap in enumerate(pieces):
                        nc.tensor.matmul(h1_ps[row:row + nrow, :],
                                         lhsT=ap, rhs=se1_sb,
                                         start=(pi == 0),
                                         stop=(pi == len(pieces) - 1))
                h1n = groupnorm_rows(h1_ps, rows, HID, gn1s_sb, gn1b_sb,
                                     f"{tag}gn1")
                # mish(v) = v * tanh(softplus(v)) = v * (1 - 2/((1+e^v)^2+1))
                ev = spool.tile([rows, HID], f32, tag=f"{tag}m_ev",
                                name=f"{tag}m_ev")
                nc.scalar.activation(ev[:], h1n[:], Act.Exp)
                ep1 = spool.tile([rows, HID], f32, tag=f"{tag}m_ep1",
                                 name=f"{tag}m_ep1")
                nc.vector.tensor_scalar_add(ep1[:], ev[:], 1.0)
                q = spool.tile([rows, HID], f32, tag=f"{tag}m_q",
                               name=f"{tag}m_q")
                nc.vector.tensor_mul(q[:], ep1[:], ep1[:])
                qp1 = spool.tile([rows, HID], f32, tag=f"{tag}m_qp1",
                                 name=f"{tag}m_qp1")
                nc.vector.tensor_scalar_add(qp1[:], q[:], 1.0)
                rq = spool.tile([rows, HID], f32, tag=f"{tag}m_rq",
                                name=f"{tag}m_rq")
                nc.vector.reciprocal(rq[:], qp1[:])
                th = spool.tile([rows, HID], f32, tag=f"{tag}m_th",
                                name=f"{tag}m_th")
                nc.vector.tensor_scalar(th[:], rq[:], -2.0, 1.0,
                                        op0=Alu.mult, op1=Alu.add)
                h1m = spool.tile([rows, HID], f32, tag=f"{tag}m_out",
                                 name=f"{tag}m_out")
                nc.vector.tensor_mul(h1m[:], h1n[:], th[:])

                # h2[r, k] = sum_h h1m[r, h] * se_w2[k, h]: one broadcast
                # multiply ([rows,K,HID] with h1m read stride-0 over K) and
                # one innermost-axis reduce
                hk = spool.tile([rows, K * HID], f32, tag=f"{tag}hk",
                                name=f"{tag}hk")
                nc.vector.tensor_mul(
                    hk[:].rearrange("p (k h) -> p k h", k=K),
                    h1m[:].unsqueeze(1).broadcast_to([rows, K, HID]),
                    se2_bc[0:rows, :].rearrange("p (k h) -> p k h", k=K))
                h2 = spool.tile([rows, K], f32, tag=f"{tag}h2",
                                name=f"{tag}h2")
                nc.vector.tensor_reduce(
                    out=h2[:], in_=hk[:].rearrange("p (k h) -> p k h", k=K),
                    axis=mybir.AxisListType.X, op=Alu.add)
                h2n = groupnorm_rows(h2, rows, K, gn2s_sb, gn2b_sb,
                                     f"{tag}gn2")
                # sigmoid(z) = 0.5 * (1 + tanh(z/2))
                tnh = spool.tile([rows, K], f32, tag=f"{tag}a_tnh",
                                 name=f"{tag}a_tnh")
                nc.scalar.activation(tnh[:], h2n[:], Act.Tanh, scale=0.5)
                att = spool.tile([rows, K], f32, tag=f"{tag}a_att",
                                 name=f"{tag}a_att")
                ai = nc.vector.tensor_scalar(att[:], tnh[:], 0.5, 0.5,
                                             op0=Alu.mult, op1=Alu.add)
                return att, ai

            def aggb_from(att_bc, off, tag):
                """agg_b.T column: sum_k att[k] * bias[k, co], on DVE."""
                tmp = spool.tile([C, K], f32, tag=f"{tag}gbt",
                                 name=f"{tag}gbt")
                nc.vector.tensor_mul(tmp[:], biasT_sb,
                                     att_bc[:, off * K:(off + 1) * K])
                aggb = spool.tile([C, 1], f32, tag=f"{tag}gb",
                                  name=f"{tag}gb")
                nc.vector.tensor_reduce(out=aggb[:], in_=tmp[:],
                                        axis=mybir.AxisListType.X, op=Alu.add)
                return aggb

            def aggregate(b, att_bc, att_off, chunks):
                cw = W1 // chunks
                order = (1, 0, 2) if chunks == 3 else range(chunks)
                out_by_c = {}
                for c in order:
                    prev = None
                    for k in range(K):
                        cur = apool.tile([C, cw], bf16,
                                         tag=f"agg{b % 2}_{c}_{k}",
                                         name=f"agg{b}_{c}_{k}")
                        sc = att_bc[:, att_off * K + k:att_off * K + k + 1]
                        wk = wt_all[:, k * W1 + c * cw:k * W1 + (c + 1) * cw]
                        if prev is None:
                            nc.vector.tensor_scalar_mul(cur[:], wk, sc)
                        else:
                            nc.vector.scalar_tensor_tensor(
                                out=cur[:], in0=wk, scalar=sc, in1=prev[:],
                                op0=Alu.mult, op1=Alu.add)
                        prev = cur
                    out_by_c[c] = prev

                def agg_tap(tap):
                    c, r = divmod(tap * C, cw)
                    return out_by_c[c][:, r:r + C]
                return agg_tap

            def conv_sample(b, agg_tap, aggb, bl=0):
                for blk in range(NBLK):
                    h0 = blk * RB
                    ps = cpool.tile([C, RB * WW], f32, tag="convps",
                                    name=f"cps{b}_{blk}")
                    ti = 0
                    for dh in (0, -1, 1):
                        for dw in (-1, 0, 1):
                            tt = 1 if h0 + dh < 0 else 0
                            bt = 1 if h0 + RB - 1 + dh > HH - 1 else 0
                            nr = RB - tt - bt
                            tap = (dh + 1) * 3 + (dw + 1)
                            start = 1 + (h0 + tt + dh) * LROW + dw
                            rhs = (x_tiles[b][:, start:start + nr * LROW]
                                   .rearrange("p (r c) -> p r c", c=LROW)
                                   [:, :, 0:WW])
                            nc.tensor.matmul(
                                ps[:, tt * WW:(tt + nr) * WW],
                                lhsT=agg_tap(tap), rhs=rhs,
                                start=(ti == 0), stop=(ti == NTAP - 1))
                            ti += 1
                    osb = opool.tile([C, RB * WW], bf16, tag="osb",
                                     name=f"osb{b}_{blk}")
                    if blk % 2 == 0:
                        nc.scalar.activation(osb[:], ps[:], Act.Identity,
                                             bias=aggb[:, bl:bl + 1],
                                             scale=1.0)
                        out_eng = nc.sync
                    else:
                        nc.vector.tensor_scalar(
                            osb[:], ps[:], aggb[:, bl:bl + 1], None,
                            op0=Alu.add)
                        out_eng = nc.scalar
                    out_eng.dma_start(out_d[b][:, h0 * WW:(h0 + RB) * WW],
                                      osb[:])

            # ---- schedule: sample 0's SE alone (PE-free tail -> conv
            # starts right after it); samples 1-3 SE batched, hidden under
            # conv0. pooling: 0-2 split DVE/ACT, 3 ACT-only. ----
            pool_sample_split(0)
            att0, att0_i = se_chain(1, [(0, [p_dve0[:], p_act0[:]])], "s0")
            att_bc0 = spool.tile([C, K], f32, tag="attbc0", name="attbc0")
            nc.gpsimd.partition_broadcast(att_bc0[:], att0[:])
            aggb0 = aggb_from(att_bc0, 0, "s0")
            at0 = aggregate(0, att_bc0, 0, chunks=3)

            pool_sample1()
            pool_sample_act(2)
            pool_sample_act(3)
            att123, _ = se_chain(3, [(0, [p123[:, 0:3]])], "s123")
            att_flat = spool.tile([1, 3 * K], f32, tag="attflat",
                                  name="attflat")
            nc.scalar.dma_start(att_flat[:], att123[:])
            att_bc123 = spool.tile([C, 3 * K], f32, tag="attbc123",
                                   name="attbc123")
            nc.gpsimd.partition_broadcast(att_bc123[:], att_flat[:])

            conv_sample(0, at0, aggb0)
            aggb1 = aggb_from(att_bc123, 0, "s1")
            at1 = aggregate(1, att_bc123, 0, chunks=1)
            conv_sample(1, at1, aggb1)
            aggb2 = aggb_from(att_bc123, 1, "s2")
            at2 = aggregate(2, att_bc123, 1, chunks=1)
            conv_sample(2, at2, aggb2)
            aggb3 = aggb_from(att_bc123, 2, "s3")
            at3 = aggregate(3, att_bc123, 2, chunks=1)
            conv_sample(3, at3, aggb3)

    nc.compile()
    return nc


def _stage_inputs(x, weight, bias, se_w1, gn1_scale, gn1_bias, se_w2,
                  gn2_scale, gn2_bias):
    """Host-side layout staging: shard, pad, transpose, cast. Returns in_maps."""
    bf16 = ml_dtypes.bfloat16

    # zero-padded x: per (b, ci) buffer of length LPAD; element (h, w) lives at
    # 1 + h*LROW + w, so w-1/w+64 shifts read zeros and row shifts stay in bounds.
    xp = np.zeros((BS, C, LPAD), dtype=bf16)
    xp_view = xp[:, :, 1:1 + HH * LROW].reshape(BS, C, HH, LROW)
    xp_view[:, :, :, :WW] = x.astype(bf16)

    # weight [k, o, i, h, w] -> lhsT layout [k, i, (h*3+w)*C + o]
    wt = np.ascontiguousarray(weight.transpose(0, 2, 3, 4, 1)).reshape(K, C, NTAP * C).astype(bf16)

    # pack all small params into one [C, BLOB_W] f32 blob (single DMA issue)
    blob = np.zeros((C, BLOB_W), dtype=np.float32)
    blob[:, 0:8] = (se_w1 / float(HH * WW)).T          # se1 [C, HID]
    blob[0:K, 8:136] = bias                            # bias.T-layout [K, C]
    blob[0:HID, 136:140] = se_w2.T                     # se2 [HID, K]
    blob[0:K, 140:144] = np.eye(K, dtype=np.float32)   # eye4
    blob[0:1, 144:152] = gn1_scale.reshape(1, HID)
    blob[0:1, 152:160] = gn1_bias.reshape(1, HID)
    blob[0:1, 160:164] = gn2_scale.reshape(1, K)
    blob[0:1, 164:168] = gn2_bias.reshape(1, K)
    blob[:, 176:180] = bias.T                          # bias.T [C, K]
    blob[0:1, 200:232] = se_w2.reshape(1, K * HID)     # se2 rows, flat

    in_maps = []
    for i in range(N_CORES):
        in_maps.append({
            "xp": np.ascontiguousarray(xp[i * BSL:(i + 1) * BSL]),
            "wt": wt, "blob": blob,
        })
    return in_maps


def kernel(x, weight, bias, se_w1, gn1_scale, gn1_bias, se_w2, gn2_scale,
           gn2_bias):
    global LAST_EXEC_NS
    x = np.asarray(x, dtype=np.float32)
    weight = np.asarray(weight, dtype=np.float32)
    bias = np.asarray(bias, dtype=np.float32)
    se_w1 = np.asarray(se_w1, dtype=np.float32)
    gn1_scale = np.asarray(gn1_scale, dtype=np.float32)
    gn1_bias = np.asarray(gn1_bias, dtype=np.float32)
    se_w2 = np.asarray(se_w2, dtype=np.float32)
    gn2_scale = np.asarray(gn2_scale, dtype=np.float32)
    gn2_bias = np.asarray(gn2_bias, dtype=np.float32)

    if "nc" not in _cache:
        _cache["nc"] = _build_nc()
    nc = _cache["nc"]

    in_maps = _stage_inputs(x, weight, bias, se_w1, gn1_scale, gn1_bias,
                            se_w2, gn2_scale, gn2_bias)

    trace = bool(int(os.environ.get("BASS_KERNEL_TRACE", "0")))
    if trace:
        trace = _install_trace_hook()

    from concourse.bass_utils import run_bass_kernel_spmd
    res = run_bass_kernel_spmd(nc, in_maps, core_ids=list(range(N_CORES)),
                               trace=trace)
    LAST_EXEC_NS = res.exec_time_ns

    out = np.empty((BS, C, HH, WW), dtype=np.float32)
    for i in range(N_CORES):
        out[i * BSL:(i + 1) * BSL] = (
            res.results[i]["out"].astype(np.float32).reshape(BSL, C, HH, WW))
    return out
